# revision 1
# baseline (speedup 1.0000x reference)
"""ConvIntNet (interaction-network) Trainium2 kernel.

Strategy (pure data parallelism over batch, 8 cores x 16 batch elements):
  The dense one-hot relation einsums are algebraically removed. With edges
  ordered receiver-major, edge (r, s) has
      h1 = relu(A[r] + S[s] + eb1),  A = xn @ W1_rec, S = xn @ W1_snd
  so stage 1 is a broadcast-add + relu (per-partition-scalar ops), stages
  2/3 are block-diagonal-packed matmuls, and the receiver scatter-add is a
  segmented sum over s fused into the stage-3 relu via accum_out. Self-edge
  (s == r) contributions are computed by a small diagonal pipeline and
  subtracted. BatchNorm is folded into W1/biases on the host.

Layout per batch element:
  nodes padded 150 -> 168 = 4 groups x 42; partition dim carries
  4 x (30|15|6)-feature groups; free dim carries (q, s) edge positions in
  14 chunks of 450 = 3 q-blocks x 150 senders.
"""

import numpy as np

import concourse.bacc as bacc
import concourse.tile as tile
from concourse import mybir

f32 = mybir.dt.float32
bf16 = mybir.dt.bfloat16
Alu = mybir.AluOpType
Act = mybir.ActivationFunctionType
AxX = mybir.AxisListType.X

# ---- problem dims (hardcoded per contract) ----
B, N, F = 128, 150, 16
NCORES = 8
BL = B // NCORES          # 16 batch elements per core
EH, EH2, NEFF = 30, 15, 6
DH, DH2, NDYN = 45, 22, 6
ABS, NCLS = 48, 5
BN_EPS = 1e-3
NP = 168                  # padded nodes = 4 * 42
QG = NP // 4              # 42 q positions per partition group
NCH = 14                  # chunks of 450 = 3 q-blocks
CHW = 450

# h1 engine split: q -> engine
H1_DVE = set(range(0, 24))
H1_GP = set(range(24, 38))
H1_ACT = set(range(38, 42))


def _build_module():
    nc = bacc.Bacc("TRN2", target_bir_lowering=False)

    xt_d = nc.dram_tensor("xt", [BL, F, NP], f32, kind="ExternalInput")
    wr_d = nc.dram_tensor("wr", [F, 32], bf16, kind="ExternalInput")
    ws_d = nc.dram_tensor("ws", [F, 128], bf16, kind="ExternalInput")
    w2_d = nc.dram_tensor("w2", [128, 64], bf16, kind="ExternalInput")
    w3_d = nc.dram_tensor("w3", [128, 64], bf16, kind="ExternalInput")
    w3s_d = nc.dram_tensor("w3s", [64, 32], bf16, kind="ExternalInput")
    zst_d = nc.dram_tensor("zst", [24, 4 * DH], f32, kind="ExternalInput")
    w1x_d = nc.dram_tensor("w1x", [F, DH], f32, kind="ExternalInput")
    wd2_d = nc.dram_tensor("wd2", [DH, DH2], f32, kind="ExternalInput")
    wd3_d = nc.dram_tensor("wd3", [DH2, NDYN], f32, kind="ExternalInput")
    wa1_d = nc.dram_tensor("wa1", [NDYN, ABS], f32, kind="ExternalInput")
    wa2_d = nc.dram_tensor("wa2", [ABS + 1, NCLS], f32, kind="ExternalInput")
    bia_d = nc.dram_tensor("bia", [128, 11], f32, kind="ExternalInput")
    bab_d = nc.dram_tensor("bab", [128, 2], f32, kind="ExternalInput")
    ones_d = nc.dram_tensor("ones", [1, BL], f32, kind="ExternalInput")
    y_d = nc.dram_tensor("y", [BL, NCLS], f32, kind="ExternalOutput")

    from contextlib import ExitStack
    ctx = ExitStack()
    with tile.TileContext(nc) as tc, ctx:
        consts = ctx.enter_context(tc.tile_pool(name="consts", bufs=1))
        xp = ctx.enter_context(tc.tile_pool(name="xp", bufs=2))
        asp = ctx.enter_context(tc.tile_pool(name="asp", bufs=2))
        h1p = ctx.enter_context(tc.tile_pool(name="h1p", bufs=2))
        h2p = ctx.enter_context(tc.tile_pool(name="h2p", bufs=3))
        effp = ctx.enter_context(tc.tile_pool(name="effp", bufs=3))
        rp = ctx.enter_context(tc.tile_pool(name="rp", bufs=8))
        smp = ctx.enter_context(tc.tile_pool(name="smp", bufs=3))
        psAS = ctx.enter_context(tc.tile_pool(name="psAS", bufs=2, space="PSUM"))
        ps2 = ctx.enter_context(tc.tile_pool(name="ps2", bufs=2, space="PSUM"))
        ps3 = ctx.enter_context(tc.tile_pool(name="ps3", bufs=2, space="PSUM"))
        psD = ctx.enter_context(tc.tile_pool(name="psD", bufs=2, space="PSUM"))

        # ---- load constants ----
        wr_t = consts.tile([F, 32], bf16)
        ws_t = consts.tile([F, 128], bf16)
        w2_t = consts.tile([128, 64], bf16)
        w3_t = consts.tile([128, 64], bf16)
        w3s_t = consts.tile([64, 32], bf16)
        zst_t = consts.tile([24, 4 * DH], f32)
        w1x_t = consts.tile([F, DH], f32)
        wd2_t = consts.tile([DH, DH2], f32)
        wd3_t = consts.tile([DH2, NDYN], f32)
        wa1_t = consts.tile([NDYN, ABS], f32)
        wa2_t = consts.tile([ABS + 1, NCLS], f32)
        bia_t = consts.tile([128, 11], f32)
        bab_t = consts.tile([128, 2], f32)
        for t, d in [(wr_t, wr_d), (ws_t, ws_d), (w2_t, w2_d), (w3_t, w3_d),
                     (w3s_t, w3s_d), (zst_t, zst_d), (w1x_t, w1x_d),
                     (wd2_t, wd2_d), (wd3_t, wd3_d), (wa1_t, wa1_d),
                     (wa2_t, wa2_d), (bia_t, bia_d), (bab_t, bab_d)]:
            nc.sync.dma_start(out=t, in_=d.ap())

        bA = bab_t[:, 0:1]          # bf16 A-side bias (incl eb1), 4x30 pattern
        bS = bab_t[:, 1:2]          # bf16 S-side bias
        eb2r = bia_t[:, 2:3]        # fp32, rows 64u+15j+f2
        eb3r = bia_t[:, 3:4]        # fp32, rows 64g+32u+6j+c
        eb3d = bia_t[0:32, 4:5]
        db1 = bia_t[0:DH, 5:6]
        db2 = bia_t[0:DH2, 6:7]
        db3 = bia_t[0:NDYN, 7:8]
        ab1 = bia_t[0:ABS, 8:9]
        sc_pp = bia_t[0:F, 9:10]
        sh_pp = bia_t[0:F, 10:11]

        pooled = consts.tile([NDYN, BL], f32)

        for b in range(BL):
            x_t = xp.tile([F, NP], f32, tag="x_t")
            nc.sync.dma_start(out=x_t, in_=xt_d.ap()[b])

            x_tb = xp.tile([F, NP], bf16, tag="x_tb")
            nc.vector.tensor_copy(out=x_tb, in_=x_t)

            # A[32j+f, q] = xn[42j+q] @ W1r'  (4 matmuls, M=32 col-tiles)
            a_ps_full = psAS.tile([128, 512], f32, tag="as")
            a_ps = a_ps_full[:, 0:QG]
            for j in range(4):
                nc.tensor.matmul(
                    out=a_ps[32 * j:32 * j + 32, :], lhsT=wr_t,
                    rhs=x_tb[:, QG * j:QG * (j + 1)],
                    start=True, stop=True, tile_position=(0, 32 * j))
            a_s = asp.tile([128, QG], f32, tag="a_s")
            nc.scalar.activation(out=a_s, in_=a_ps, func=Act.Identity, bias=bA)
            a_sb = asp.tile([128, QG], bf16, tag="a_sb")
            nc.vector.tensor_copy(out=a_sb, in_=a_s)

            # S_rep[32j+f, s] = xn[s] @ W1s' (one matmul, 4x replicated lhsT)
            s_ps_full = psAS.tile([128, 512], f32, tag="as")
            s_ps = s_ps_full[:, 0:NP]
            nc.tensor.matmul(out=s_ps, lhsT=ws_t, rhs=x_tb, start=True, stop=True)
            s_rep = asp.tile([128, NP], bf16, tag="s_rep")
            nc.scalar.activation(out=s_rep, in_=s_ps, func=Act.Identity, bias=bS)

            # h1[p, q*150+s] = relu(S_rep[p, s] + A[p, q])
            h1 = h1p.tile([128, NCH * CHW], bf16, tag="h1")
            for q in range(QG):
                dst = h1[:, q * N:(q + 1) * N]
                if q in H1_DVE:
                    nc.vector.tensor_scalar(
                        out=dst, in0=s_rep[:, 0:N], scalar1=a_s[:, q:q + 1],
                        scalar2=0.0, op0=Alu.add, op1=Alu.max)
                elif q in H1_GP:
                    nc.gpsimd.tensor_scalar(
                        out=dst, in0=s_rep[:, 0:N], scalar1=a_s[:, q:q + 1],
                        scalar2=0.0, op0=Alu.add, op1=Alu.max)
                else:
                    nc.scalar.activation(
                        out=dst, in_=s_rep[:, 0:N], func=Act.Relu,
                        bias=a_s[:, q:q + 1])

            # stage 2 + 3 + fused relu/segment-sum
            r_tiles = []
            p3_cur = None
            h2_cur = None
            p2_cur = None
            for c in range(NCH):
                half = c % 2
                if half == 0:
                    p2_full = ps2.tile([128, 512], f32, tag="p2")
                    p2_cur = p2_full[:, 0:CHW]
                nc.tensor.matmul(
                    out=p2_cur[64 * half:64 * half + 64, :], lhsT=w2_t,
                    rhs=h1[:, c * CHW:(c + 1) * CHW],
                    start=True, stop=True, tile_position=(0, 64 * half))
                if half == 1 or c == NCH - 1:
                    p = c // 2
                    rows = 128 if half == 1 else 64
                    h2_cur = h2p.tile([128, CHW], bf16, tag="h2")
                    nc.vector.tensor_scalar(
                        out=h2_cur[0:rows, 0:225], in0=p2_cur[0:rows, 0:225],
                        scalar1=eb2r[0:rows], scalar2=0.0,
                        op0=Alu.add, op1=Alu.max)
                    nc.scalar.activation(
                        out=h2_cur[0:rows, 225:450], in_=p2_cur[0:rows, 225:450],
                        func=Act.Relu, bias=eb2r[0:rows])
                    g = p % 2
                    if g == 0:
                        p3_full = ps3.tile([128, 512], f32, tag="p3")
                        p3_cur = p3_full[:, 0:CHW]
                        r_t = rp.tile([128, 3], f32, tag="r")
                        r_tiles.append(r_t)
                    nc.tensor.matmul(
                        out=p3_cur[64 * g:64 * g + 64, 0:CHW], lhsT=w3_t,
                        rhs=h2_cur, start=True, stop=True,
                        tile_position=(0, 64 * g))
                    if g == 1 or p == 6:
                        rows3 = 128 if g == 1 else 64
                        r_t = r_tiles[-1]
                        for i in range(3):
                            scr = effp.tile([128, N], bf16, tag="escr")
                            src = p3_cur[0:rows3, i * N:(i + 1) * N]
                            if i % 2 == 0:
                                nc.vector.tensor_scalar(
                                    out=scr[0:rows3, :], in0=src,
                                    scalar1=eb3r[0:rows3], scalar2=0.0,
                                    op0=Alu.add, op1=Alu.max,
                                    accum_out=r_t[0:rows3, i:i + 1])
                            else:
                                nc.scalar.activation(
                                    out=scr[0:rows3, :], in_=src, func=Act.Relu,
                                    bias=eb3r[0:rows3],
                                    accum_out=r_t[0:rows3, i:i + 1])

            # gather segment sums -> EffR[6j+c', q]
            effr = smp.tile([24, QG], f32, tag="effr")
            for c in range(NCH):
                w_, g_, u_ = c // 4, (c % 4) // 2, c % 2
                nc.sync.dma_start(
                    out=effr[:, 3 * c:3 * c + 3],
                    in_=r_tiles[w_][64 * g_ + 32 * u_:64 * g_ + 32 * u_ + 24, :])

            # diagonal (self-edge) pipeline
            pd = smp.tile([128, QG], bf16, tag="pd")
            for j in range(4):
                nc.vector.tensor_tensor(
                    out=pd[32 * j:32 * (j + 1), :],
                    in0=a_sb[32 * j:32 * (j + 1), :],
                    in1=s_rep[32 * j:32 * (j + 1), QG * j:QG * (j + 1)],
                    op=Alu.add)
            pdr = smp.tile([128, QG], bf16, tag="pdr")
            nc.vector.tensor_scalar_max(out=pdr, in0=pd, scalar1=0.0)
            pd2_full = psD.tile([64, 512], f32, tag="dps")
            pd2 = pd2_full[:, 0:QG]
            nc.tensor.matmul(out=pd2, lhsT=w2_t, rhs=pdr, start=True, stop=True)
            h2d = smp.tile([64, QG], bf16, tag="h2d")
            nc.scalar.activation(out=h2d, in_=pd2, func=Act.Relu, bias=eb2r[0:64])
            pd3_full = psD.tile([32, 512], f32, tag="dps")
            pd3 = pd3_full[:, 0:QG]
            nc.tensor.matmul(out=pd3, lhsT=w3s_t, rhs=h2d, start=True, stop=True)
            eself = smp.tile([32, QG], f32, tag="eself")
            nc.scalar.activation(out=eself, in_=pd3, func=Act.Relu, bias=eb3d)

            effrf = smp.tile([24, QG], f32, tag="effrf")
            nc.vector.tensor_tensor(out=effrf, in0=effr, in1=eself[0:24, :],
                                    op=Alu.subtract)

            # dynamics MLP; node sum fused into last relu
            xn_t = xp.tile([F, NP], f32, tag="xn_t")
            nc.vector.tensor_scalar(out=xn_t, in0=x_t, scalar1=sc_pp,
                                    scalar2=sh_pp, op0=Alu.mult, op1=Alu.add)
            d1a_full = psD.tile([DH, 512], f32, tag="dps")
            d1a = d1a_full[:, 0:N]
            nc.tensor.matmul(out=d1a, lhsT=w1x_t, rhs=xn_t[:, 0:N],
                             start=True, stop=True)
            d1e_full = psD.tile([DH, 512], f32, tag="dps")
            d1e = d1e_full[:, 0:N]
            for j in range(4):
                cnt = min(QG, N - QG * j)
                nc.tensor.matmul(
                    out=d1e[:, QG * j:QG * j + cnt],
                    lhsT=zst_t[:, DH * j:DH * (j + 1)],
                    rhs=effrf[:, 0:cnt], start=True, stop=True)
            d1es = smp.tile([DH, N], f32, tag="d1es")
            nc.vector.tensor_scalar(out=d1es, in0=d1e, scalar1=db1,
                                    scalar2=None, op0=Alu.add)
            d1sum = smp.tile([DH, N], f32, tag="d1sum")
            nc.vector.tensor_tensor(out=d1sum, in0=d1a, in1=d1es, op=Alu.add)
            d1s = smp.tile([DH, N], f32, tag="d1s")
            nc.scalar.activation(out=d1s, in_=d1sum, func=Act.Relu)
            d2_full = psD.tile([DH2, 512], f32, tag="dps")
            d2 = d2_full[:, 0:N]
            nc.tensor.matmul(out=d2, lhsT=wd2_t, rhs=d1s, start=True, stop=True)
            d2s = smp.tile([DH2, N], f32, tag="d2s")
            nc.scalar.activation(out=d2s, in_=d2, func=Act.Relu, bias=db2)
            d3_full = psD.tile([NDYN, 512], f32, tag="dps")
            d3 = d3_full[:, 0:N]
            nc.tensor.matmul(out=d3, lhsT=wd3_t, rhs=d2s, start=True, stop=True)
            dyn_s = smp.tile([NDYN, N], f32, tag="dyn_s")
            nc.scalar.activation(out=dyn_s, in_=d3, func=Act.Relu, bias=db3,
                                 accum_out=pooled[:, b:b + 1])

        # abstract MLP + softmax (once per core)
        pa_full = psD.tile([ABS, 512], f32, tag="dps")
        pa = pa_full[:, 0:BL]
        nc.tensor.matmul(out=pa, lhsT=wa1_t, rhs=pooled, start=True, stop=True)
        ha = consts.tile([ABS + 1, BL], f32)
        nc.scalar.activation(out=ha[0:ABS, :], in_=pa, func=Act.Relu, bias=ab1)
        nc.sync.dma_start(out=ha[ABS:ABS + 1, :], in_=ones_d.ap())
        zl_full = psD.tile([BL, 512], f32, tag="dps")
        zl = zl_full[:, 0:NCLS]
        nc.tensor.matmul(out=zl, lhsT=ha, rhs=wa2_t, start=True, stop=True)
        ex = consts.tile([BL, NCLS], f32)
        nc.scalar.activation(out=ex, in_=zl, func=Act.Exp)
        ssum = consts.tile([BL, 1], f32)
        nc.vector.tensor_reduce(out=ssum, in_=ex, axis=AxX, op=Alu.add)
        rcp = consts.tile([BL, 1], f32)
        nc.vector.reciprocal(out=rcp, in_=ssum)
        outt = consts.tile([BL, NCLS], f32)
        nc.vector.tensor_scalar_mul(out=outt, in0=ex, scalar1=rcp)
        nc.sync.dma_start(out=y_d.ap(), in_=outt)

    nc.compile()
    return nc


def _prep_consts(inp):
    """Host-side weight preprocessing (tiny, O(KB))."""
    g = lambda k: np.asarray(inp[k], np.float32)
    sc = g("bn_gamma") / np.sqrt(g("bn_var") + BN_EPS)
    sh = g("bn_beta") - g("bn_mean") * sc
    W1 = g("eW1")
    W1r = sc[:, None] * W1[:F]
    W1s = sc[:, None] * W1[F:]
    bA = sh @ W1[:F] + g("eb1")
    bS = sh @ W1[F:]

    wr = np.zeros((F, 32), np.float32)
    wr[:, :EH] = W1r
    ws = np.zeros((F, 128), np.float32)
    for j in range(4):
        ws[:, 32 * j:32 * j + EH] = W1s

    w2 = np.zeros((128, 64), np.float32)
    eW2 = g("eW2")
    for j in range(4):
        w2[32 * j:32 * j + EH, 15 * j:15 * j + EH2] = eW2
    w3 = np.zeros((128, 64), np.float32)
    eW3 = g("eW3")
    for u in range(2):
        for j in range(4):
            w3[64 * u + 15 * j:64 * u + 15 * j + EH2,
               32 * u + 6 * j:32 * u + 6 * j + NEFF] = eW3
    w3s = np.zeros((64, 32), np.float32)
    for j in range(4):
        w3s[15 * j:15 * j + EH2, 6 * j:6 * j + NEFF] = eW3

    dW1 = g("dW1")
    zst = np.zeros((24, 4 * DH), np.float32)
    for j in range(4):
        zst[6 * j:6 * j + NEFF, DH * j:DH * (j + 1)] = dW1[F:F + NEFF]

    wa2 = np.vstack([g("aW2"), g("ab2")[None, :]]).astype(np.float32)

    bia = np.zeros((128, 11), np.float32)
    bab = np.zeros((128, 2), np.float32)
    for j in range(4):
        bab[32 * j:32 * j + EH, 0] = bA
        bab[32 * j:32 * j + EH, 1] = bS
        bia[15 * j:15 * j + EH2, 2] = g("eb2")
        bia[64 + 15 * j:64 + 15 * j + EH2, 2] = g("eb2")
        bia[6 * j:6 * j + NEFF, 4] = g("eb3")
        for gg in range(2):
            for u in range(2):
                bia[64 * gg + 32 * u + 6 * j:64 * gg + 32 * u + 6 * j + NEFF,
                    3] = g("eb3")
    bia[0:DH, 5] = g("db1")
    bia[0:DH2, 6] = g("db2")
    bia[0:NDYN, 7] = g("db3")
    bia[0:ABS, 8] = g("ab1")
    bia[0:F, 9] = sc
    bia[0:F, 10] = sh

    import ml_dtypes
    tobf = lambda a: np.asarray(a, np.float32).astype(ml_dtypes.bfloat16)
    return {
        "wr": tobf(wr), "ws": tobf(ws), "w2": tobf(w2), "w3": tobf(w3),
        "w3s": tobf(w3s), "zst": zst, "w1x": dW1[:F].astype(np.float32),
        "wd2": g("dW2"), "wd3": g("dW3"), "wa1": g("aW1"), "wa2": wa2,
        "bia": bia, "bab": bab, "ones": np.ones((1, BL), np.float32),
    }


def _prep_xt(x):
    """x (B, N, F) -> per-core transposed/padded (NCORES, BL, F, NP)."""
    x = np.asarray(x, np.float32)
    xt = np.zeros((B, F, NP), np.float32)
    xt[:, :, :N] = np.transpose(x, (0, 2, 1))
    return xt.reshape(NCORES, BL, F, NP)


_NC_CACHE = {}


def _get_module():
    if "nc" not in _NC_CACHE:
        _NC_CACHE["nc"] = _build_module()
    return _NC_CACHE["nc"]


def make_in_maps(inputs):
    consts = _prep_consts(inputs)
    xt = _prep_xt(inputs["x"])
    return [dict(consts, xt=np.ascontiguousarray(xt[c])) for c in range(NCORES)]


def kernel(**inputs) -> np.ndarray:
    from concourse.bass_utils import run_bass_kernel_spmd
    nc = _get_module()
    in_maps = make_in_maps(inputs)
    res = run_bass_kernel_spmd(nc, in_maps, core_ids=list(range(NCORES)))
    return np.concatenate([r["y"] for r in res.results], axis=0)



# revision 6
# speedup vs baseline: 21.5335x; 21.5335x over previous
"""ConvIntNet (interaction-network) Trainium2 kernel.

Strategy (pure data parallelism over batch, 8 cores x 16 batch elements):
  The dense one-hot relation einsums are algebraically removed. With edges
  ordered receiver-major, edge (r, s) has
      h1 = relu(A[r] + S[s] + eb1),  A = xn @ W1_rec, S = xn @ W1_snd
  so stage 1 is a broadcast-add + relu (per-partition-scalar ops), stages
  2/3 are block-diagonal-packed matmuls, and the receiver scatter-add is a
  segmented sum over s fused into the stage-3 relu via accum_out. Self-edge
  (s == r) contributions are computed by a small diagonal pipeline and
  subtracted. BatchNorm is folded into W1/biases on the host.

Layout per batch element:
  nodes padded 150 -> 168 = 4 groups x 42; partition dim carries
  4 x (30|15|6)-feature groups; free dim carries (q, s) edge positions in
  14 chunks of 450 = 3 q-blocks x 150 senders.
"""

import numpy as np

import concourse.bacc as bacc
import concourse.tile as tile
from concourse import mybir
from concourse.bass import ds, ts

f32 = mybir.dt.float32
bf16 = mybir.dt.bfloat16
Alu = mybir.AluOpType
Act = mybir.ActivationFunctionType
AxX = mybir.AxisListType.X

# ---- problem dims (hardcoded per contract) ----
B, N, F = 128, 150, 16
NCORES = 8
BL = B // NCORES          # 16 batch elements per core
EH, EH2, NEFF = 30, 15, 6
DH, DH2, NDYN = 45, 22, 6
ABS, NCLS = 48, 5
BN_EPS = 1e-3
NP = 168                  # padded nodes = 4 * 42
QG = NP // 4              # 42 q positions per partition group
NCH = 14                  # chunks of 450 = 3 q-blocks
CHW = 450

# h1 engine split: q -> engine
H1_DVE = set(range(0, 24))
H1_GP = set(range(24, 38))
H1_ACT = set(range(38, 42))


def _build_module():
    nc = bacc.Bacc("TRN2", target_bir_lowering=False)

    xt_d = nc.dram_tensor("xt", [BL * F, NP], f32, kind="ExternalInput")
    wr_d = nc.dram_tensor("wr", [F, 32], bf16, kind="ExternalInput")
    ws_d = nc.dram_tensor("ws", [F, 128], bf16, kind="ExternalInput")
    w2_d = nc.dram_tensor("w2", [128, 64], bf16, kind="ExternalInput")
    w3_d = nc.dram_tensor("w3", [128, 64], bf16, kind="ExternalInput")
    w3s_d = nc.dram_tensor("w3s", [64, 32], bf16, kind="ExternalInput")
    zst_d = nc.dram_tensor("zst", [24, 4 * DH], f32, kind="ExternalInput")
    w1x_d = nc.dram_tensor("w1x", [F, DH], f32, kind="ExternalInput")
    wd2_d = nc.dram_tensor("wd2", [DH, DH2], f32, kind="ExternalInput")
    wd3_d = nc.dram_tensor("wd3", [DH2, NDYN], f32, kind="ExternalInput")
    wa1_d = nc.dram_tensor("wa1", [NDYN, ABS], f32, kind="ExternalInput")
    wa2_d = nc.dram_tensor("wa2", [ABS + 1, NCLS], f32, kind="ExternalInput")
    bia_d = nc.dram_tensor("bia", [128, 11], f32, kind="ExternalInput")
    bab_d = nc.dram_tensor("bab", [128, 2], f32, kind="ExternalInput")
    ones_d = nc.dram_tensor("ones", [1, BL], f32, kind="ExternalInput")
    y_d = nc.dram_tensor("y", [BL, NCLS], f32, kind="ExternalOutput")

    from contextlib import ExitStack
    ctx = ExitStack()
    with tile.TileContext(nc) as tc, ctx:
        consts = ctx.enter_context(tc.tile_pool(name="consts", bufs=1))
        xp = ctx.enter_context(tc.tile_pool(name="xp", bufs=2))
        asp = ctx.enter_context(tc.tile_pool(name="asp", bufs=2))
        h1p = ctx.enter_context(tc.tile_pool(name="h1p", bufs=2))
        h2p = ctx.enter_context(tc.tile_pool(name="h2p", bufs=3))
        effp = ctx.enter_context(tc.tile_pool(name="effp", bufs=3))
        rp = ctx.enter_context(tc.tile_pool(name="rp", bufs=8))
        smp = ctx.enter_context(tc.tile_pool(name="smp", bufs=3))
        psAS = ctx.enter_context(tc.tile_pool(name="psAS", bufs=2, space="PSUM"))
        ps2 = ctx.enter_context(tc.tile_pool(name="ps2", bufs=2, space="PSUM"))
        ps3 = ctx.enter_context(tc.tile_pool(name="ps3", bufs=2, space="PSUM"))
        psD = ctx.enter_context(tc.tile_pool(name="psD", bufs=2, space="PSUM"))

        # ---- load constants ----
        wr_t = consts.tile([F, 32], bf16)
        ws_t = consts.tile([F, 128], bf16)
        w2_t = consts.tile([128, 64], bf16)
        w3_t = consts.tile([128, 64], bf16)
        w3s_t = consts.tile([64, 32], bf16)
        zst_t = consts.tile([24, 4 * DH], f32)
        w1x_t = consts.tile([F, DH], f32)
        wd2_t = consts.tile([DH, DH2], f32)
        wd3_t = consts.tile([DH2, NDYN], f32)
        wa1_t = consts.tile([NDYN, ABS], f32)
        wa2_t = consts.tile([ABS + 1, NCLS], f32)
        bia_t = consts.tile([128, 11], f32)
        bab_t = consts.tile([128, 2], f32)
        for t, d in [(wr_t, wr_d), (ws_t, ws_d), (w2_t, w2_d), (w3_t, w3_d),
                     (w3s_t, w3s_d), (zst_t, zst_d), (w1x_t, w1x_d),
                     (wd2_t, wd2_d), (wd3_t, wd3_d), (wa1_t, wa1_d),
                     (wa2_t, wa2_d), (bia_t, bia_d), (bab_t, bab_d)]:
            nc.sync.dma_start(out=t, in_=d.ap())

        bA = bab_t[:, 0:1]          # bf16 A-side bias (incl eb1), 4x30 pattern
        bS = bab_t[:, 1:2]          # bf16 S-side bias
        eb2r = bia_t[:, 2:3]        # fp32, rows 64u+15j+f2
        eb3r = bia_t[:, 3:4]        # fp32, rows 64g+32u+6j+c
        eb3d = bia_t[0:32, 4:5]
        db1 = bia_t[0:DH, 5:6]
        db2 = bia_t[0:DH2, 6:7]
        db3 = bia_t[0:NDYN, 7:8]
        ab1 = bia_t[0:ABS, 8:9]
        sc_pp = bia_t[0:F, 9:10]
        sh_pp = bia_t[0:F, 10:11]

        pooled = consts.tile([NDYN, BL], f32)

        with tc.For_i(0, BL, 1) as b:
            x_t = xp.tile([F, NP], f32, tag="x_t")
            nc.sync.dma_start(out=x_t, in_=xt_d[ts(b, F)])

            x_tb = xp.tile([F, NP], bf16, tag="x_tb")
            nc.vector.tensor_copy(out=x_tb, in_=x_t)

            # A[32j+f, q] = xn[42j+q] @ W1r'  (4 matmuls, M=32 col-tiles)
            a_ps_full = psAS.tile([128, 512], f32, tag="as")
            a_ps = a_ps_full[:, 0:QG]
            for j in range(4):
                nc.tensor.matmul(
                    out=a_ps[32 * j:32 * j + 32, :], lhsT=wr_t,
                    rhs=x_tb[:, QG * j:QG * (j + 1)],
                    start=True, stop=True, tile_position=(0, 32 * j))
            a_s = asp.tile([128, QG], f32, tag="a_s")
            nc.scalar.activation(out=a_s, in_=a_ps, func=Act.Identity, bias=bA)
            a_sb = asp.tile([128, QG], bf16, tag="a_sb")
            nc.vector.tensor_copy(out=a_sb, in_=a_s)

            # S_rep[32j+f, s] = xn[s] @ W1s' (one matmul, 4x replicated lhsT)
            s_ps_full = psAS.tile([128, 512], f32, tag="as")
            s_ps = s_ps_full[:, 0:NP]
            nc.tensor.matmul(out=s_ps, lhsT=ws_t, rhs=x_tb, start=True, stop=True)
            s_rep = asp.tile([128, NP], bf16, tag="s_rep")
            nc.scalar.activation(out=s_rep, in_=s_ps, func=Act.Identity, bias=bS)

            # h1[p, q*150+s] = relu(S_rep[p, s] + A[p, q])
            h1 = h1p.tile([128, NCH * CHW], bf16, tag="h1")
            for q in range(QG):
                dst = h1[:, q * N:(q + 1) * N]
                if q in H1_DVE:
                    nc.vector.tensor_scalar(
                        out=dst, in0=s_rep[:, 0:N], scalar1=a_s[:, q:q + 1],
                        scalar2=0.0, op0=Alu.add, op1=Alu.max)
                elif q in H1_GP:
                    nc.gpsimd.tensor_scalar(
                        out=dst, in0=s_rep[:, 0:N], scalar1=a_s[:, q:q + 1],
                        scalar2=0.0, op0=Alu.add, op1=Alu.max)
                else:
                    nc.scalar.activation(
                        out=dst, in_=s_rep[:, 0:N], func=Act.Relu,
                        bias=a_s[:, q:q + 1])

            # stage 2 + 3 + fused relu/segment-sum
            r_tiles = []
            p3_cur = None
            h2_cur = None
            p2_cur = None
            for c in range(NCH):
                half = c % 2
                if half == 0:
                    p2_full = ps2.tile([128, 512], f32, tag="p2")
                    p2_cur = p2_full[:, 0:CHW]
                nc.tensor.matmul(
                    out=p2_cur[64 * half:64 * half + 64, :], lhsT=w2_t,
                    rhs=h1[:, c * CHW:(c + 1) * CHW],
                    start=True, stop=True, tile_position=(0, 64 * half))
                if half == 1 or c == NCH - 1:
                    p = c // 2
                    rows = 128 if half == 1 else 64
                    h2_cur = h2p.tile([128, CHW], bf16, tag="h2")
                    nc.vector.tensor_scalar(
                        out=h2_cur[0:rows, 0:225], in0=p2_cur[0:rows, 0:225],
                        scalar1=eb2r[0:rows], scalar2=0.0,
                        op0=Alu.add, op1=Alu.max)
                    nc.scalar.activation(
                        out=h2_cur[0:rows, 225:450], in_=p2_cur[0:rows, 225:450],
                        func=Act.Relu, bias=eb2r[0:rows])
                    g = p % 2
                    if g == 0:
                        p3_full = ps3.tile([128, 512], f32, tag="p3")
                        p3_cur = p3_full[:, 0:CHW]
                        r_t = rp.tile([128, 3], f32, tag="r")
                        r_tiles.append(r_t)
                    nc.tensor.matmul(
                        out=p3_cur[64 * g:64 * g + 64, 0:CHW], lhsT=w3_t,
                        rhs=h2_cur, start=True, stop=True,
                        tile_position=(0, 64 * g))
                    if g == 1 or p == 6:
                        rows3 = 128 if g == 1 else 64
                        r_t = r_tiles[-1]
                        for i in range(3):
                            scr = effp.tile([128, N], bf16, tag="escr")
                            src = p3_cur[0:rows3, i * N:(i + 1) * N]
                            if i % 2 == 0:
                                nc.vector.tensor_scalar(
                                    out=scr[0:rows3, :], in0=src,
                                    scalar1=eb3r[0:rows3], scalar2=0.0,
                                    op0=Alu.add, op1=Alu.max,
                                    accum_out=r_t[0:rows3, i:i + 1])
                            else:
                                nc.scalar.activation(
                                    out=scr[0:rows3, :], in_=src, func=Act.Relu,
                                    bias=eb3r[0:rows3],
                                    accum_out=r_t[0:rows3, i:i + 1])

            # gather segment sums -> EffR[6j+c', q]
            effr = smp.tile([24, QG], f32, tag="effr")
            for c in range(NCH):
                w_, g_, u_ = c // 4, (c % 4) // 2, c % 2
                nc.sync.dma_start(
                    out=effr[:, 3 * c:3 * c + 3],
                    in_=r_tiles[w_][64 * g_ + 32 * u_:64 * g_ + 32 * u_ + 24, :])

            # diagonal (self-edge) pipeline
            pd = smp.tile([128, QG], bf16, tag="pd")
            for j in range(4):
                nc.vector.tensor_tensor(
                    out=pd[32 * j:32 * (j + 1), :],
                    in0=a_sb[32 * j:32 * (j + 1), :],
                    in1=s_rep[32 * j:32 * (j + 1), QG * j:QG * (j + 1)],
                    op=Alu.add)
            pdr = smp.tile([128, QG], bf16, tag="pdr")
            nc.vector.tensor_scalar_max(out=pdr, in0=pd, scalar1=0.0)
            pd2_full = psD.tile([64, 512], f32, tag="dps")
            pd2 = pd2_full[:, 0:QG]
            nc.tensor.matmul(out=pd2, lhsT=w2_t, rhs=pdr, start=True, stop=True)
            h2d = smp.tile([64, QG], bf16, tag="h2d")
            nc.scalar.activation(out=h2d, in_=pd2, func=Act.Relu, bias=eb2r[0:64])
            pd3_full = psD.tile([32, 512], f32, tag="dps")
            pd3 = pd3_full[:, 0:QG]
            nc.tensor.matmul(out=pd3, lhsT=w3s_t, rhs=h2d, start=True, stop=True)
            eself = smp.tile([32, QG], f32, tag="eself")
            nc.scalar.activation(out=eself, in_=pd3, func=Act.Relu, bias=eb3d)

            effrf = smp.tile([24, QG], f32, tag="effrf")
            nc.vector.tensor_tensor(out=effrf, in0=effr, in1=eself[0:24, :],
                                    op=Alu.subtract)

            # dynamics MLP; node sum fused into last relu
            xn_t = xp.tile([F, NP], f32, tag="xn_t")
            nc.vector.tensor_scalar(out=xn_t, in0=x_t, scalar1=sc_pp,
                                    scalar2=sh_pp, op0=Alu.mult, op1=Alu.add)
            d1a_full = psD.tile([DH, 512], f32, tag="dps")
            d1a = d1a_full[:, 0:N]
            nc.tensor.matmul(out=d1a, lhsT=w1x_t, rhs=xn_t[:, 0:N],
                             start=True, stop=True)
            d1e_full = psD.tile([DH, 512], f32, tag="dps")
            d1e = d1e_full[:, 0:N]
            for j in range(4):
                cnt = min(QG, N - QG * j)
                nc.tensor.matmul(
                    out=d1e[:, QG * j:QG * j + cnt],
                    lhsT=zst_t[:, DH * j:DH * (j + 1)],
                    rhs=effrf[:, 0:cnt], start=True, stop=True)
            d1es = smp.tile([DH, N], f32, tag="d1es")
            nc.vector.tensor_scalar(out=d1es, in0=d1e, scalar1=db1,
                                    scalar2=None, op0=Alu.add)
            d1sum = smp.tile([DH, N], f32, tag="d1sum")
            nc.vector.tensor_tensor(out=d1sum, in0=d1a, in1=d1es, op=Alu.add)
            d1s = smp.tile([DH, N], f32, tag="d1s")
            nc.scalar.activation(out=d1s, in_=d1sum, func=Act.Relu)
            d2_full = psD.tile([DH2, 512], f32, tag="dps")
            d2 = d2_full[:, 0:N]
            nc.tensor.matmul(out=d2, lhsT=wd2_t, rhs=d1s, start=True, stop=True)
            d2s = smp.tile([DH2, N], f32, tag="d2s")
            nc.scalar.activation(out=d2s, in_=d2, func=Act.Relu, bias=db2)
            d3_full = psD.tile([NDYN, 512], f32, tag="dps")
            d3 = d3_full[:, 0:N]
            nc.tensor.matmul(out=d3, lhsT=wd3_t, rhs=d2s, start=True, stop=True)
            dyn_s = smp.tile([NDYN, N], f32, tag="dyn_s")
            nc.scalar.activation(out=dyn_s, in_=d3, func=Act.Relu, bias=db3,
                                 accum_out=pooled[:, ds(b, 1)])

        # abstract MLP + softmax (once per core)
        pa_full = psD.tile([ABS, 512], f32, tag="dps")
        pa = pa_full[:, 0:BL]
        nc.tensor.matmul(out=pa, lhsT=wa1_t, rhs=pooled, start=True, stop=True)
        ha = consts.tile([ABS + 1, BL], f32)
        nc.scalar.activation(out=ha[0:ABS, :], in_=pa, func=Act.Relu, bias=ab1)
        nc.sync.dma_start(out=ha[ABS:ABS + 1, :], in_=ones_d.ap())
        zl_full = psD.tile([BL, 512], f32, tag="dps")
        zl = zl_full[:, 0:NCLS]
        nc.tensor.matmul(out=zl, lhsT=ha, rhs=wa2_t, start=True, stop=True)
        ex = consts.tile([BL, NCLS], f32)
        nc.scalar.activation(out=ex, in_=zl, func=Act.Exp)
        ssum = consts.tile([BL, 1], f32)
        nc.vector.tensor_reduce(out=ssum, in_=ex, axis=AxX, op=Alu.add)
        rcp = consts.tile([BL, 1], f32)
        nc.vector.reciprocal(out=rcp, in_=ssum)
        outt = consts.tile([BL, NCLS], f32)
        nc.vector.tensor_scalar_mul(out=outt, in0=ex, scalar1=rcp)
        nc.sync.dma_start(out=y_d.ap(), in_=outt)

    nc.compile()
    return nc


def _prep_consts(inp):
    """Host-side weight preprocessing (tiny, O(KB))."""
    g = lambda k: np.asarray(inp[k], np.float32)
    sc = g("bn_gamma") / np.sqrt(g("bn_var") + BN_EPS)
    sh = g("bn_beta") - g("bn_mean") * sc
    W1 = g("eW1")
    W1r = sc[:, None] * W1[:F]
    W1s = sc[:, None] * W1[F:]
    bA = sh @ W1[:F] + g("eb1")
    bS = sh @ W1[F:]

    wr = np.zeros((F, 32), np.float32)
    wr[:, :EH] = W1r
    ws = np.zeros((F, 128), np.float32)
    for j in range(4):
        ws[:, 32 * j:32 * j + EH] = W1s

    w2 = np.zeros((128, 64), np.float32)
    eW2 = g("eW2")
    for j in range(4):
        w2[32 * j:32 * j + EH, 15 * j:15 * j + EH2] = eW2
    w3 = np.zeros((128, 64), np.float32)
    eW3 = g("eW3")
    for u in range(2):
        for j in range(4):
            w3[64 * u + 15 * j:64 * u + 15 * j + EH2,
               32 * u + 6 * j:32 * u + 6 * j + NEFF] = eW3
    w3s = np.zeros((64, 32), np.float32)
    for j in range(4):
        w3s[15 * j:15 * j + EH2, 6 * j:6 * j + NEFF] = eW3

    dW1 = g("dW1")
    zst = np.zeros((24, 4 * DH), np.float32)
    for j in range(4):
        zst[6 * j:6 * j + NEFF, DH * j:DH * (j + 1)] = dW1[F:F + NEFF]

    wa2 = np.vstack([g("aW2"), g("ab2")[None, :]]).astype(np.float32)

    bia = np.zeros((128, 11), np.float32)
    bab = np.zeros((128, 2), np.float32)
    for j in range(4):
        bab[32 * j:32 * j + EH, 0] = bA
        bab[32 * j:32 * j + EH, 1] = bS
        bia[15 * j:15 * j + EH2, 2] = g("eb2")
        bia[64 + 15 * j:64 + 15 * j + EH2, 2] = g("eb2")
        bia[6 * j:6 * j + NEFF, 4] = g("eb3")
        for gg in range(2):
            for u in range(2):
                bia[64 * gg + 32 * u + 6 * j:64 * gg + 32 * u + 6 * j + NEFF,
                    3] = g("eb3")
    bia[0:DH, 5] = g("db1")
    bia[0:DH2, 6] = g("db2")
    bia[0:NDYN, 7] = g("db3")
    bia[0:ABS, 8] = g("ab1")
    bia[0:F, 9] = sc
    bia[0:F, 10] = sh

    import ml_dtypes
    tobf = lambda a: np.asarray(a, np.float32).astype(ml_dtypes.bfloat16)
    return {
        "wr": tobf(wr), "ws": tobf(ws), "w2": tobf(w2), "w3": tobf(w3),
        "w3s": tobf(w3s), "zst": zst, "w1x": dW1[:F].astype(np.float32),
        "wd2": g("dW2"), "wd3": g("dW3"), "wa1": g("aW1"), "wa2": wa2,
        "bia": bia, "bab": bab, "ones": np.ones((1, BL), np.float32),
    }


def _prep_xt(x):
    """x (B, N, F) -> per-core transposed/padded (NCORES, BL*F, NP)."""
    x = np.asarray(x, np.float32)
    xt = np.zeros((B, F, NP), np.float32)
    xt[:, :, :N] = np.transpose(x, (0, 2, 1))
    return xt.reshape(NCORES, BL * F, NP)


_NC_CACHE = {}


def _get_module():
    if "nc" not in _NC_CACHE:
        _NC_CACHE["nc"] = _build_module()
    return _NC_CACHE["nc"]


def make_in_maps(inputs):
    consts = _prep_consts(inputs)
    xt = _prep_xt(inputs["x"])
    return [dict(consts, xt=np.ascontiguousarray(xt[c])) for c in range(NCORES)]


def kernel(**inputs) -> np.ndarray:
    from concourse.bass_utils import run_bass_kernel_spmd
    nc = _get_module()
    in_maps = make_in_maps(inputs)
    res = run_bass_kernel_spmd(nc, in_maps, core_ids=list(range(NCORES)))
    return np.concatenate([r["y"] for r in res.results], axis=0)



# revision 8
# speedup vs baseline: 32.2268x; 1.4966x over previous
"""ConvIntNet (interaction-network) Trainium2 kernel.

Strategy (pure data parallelism over batch, 8 cores x 16 batch elements):
  The dense one-hot relation einsums are algebraically removed. With edges
  ordered receiver-major, edge (r, s) has
      h1 = relu(A[r] + S[s] + eb1),  A = xn @ W1_rec, S = xn @ W1_snd
  computed as ONE broadcast tensor_tensor op over a [128, 42*150] tile
  (4 node groups packed in the partition dim). Self-edges are removed by
  zeroing the h1 diagonal; the resulting constant edge-MLP(0) bias is
  folded into the dynamics b1 on the host. Stages 2/3 are block-diagonal
  packed matmuls; the receiver scatter-add is a segmented tensor_reduce.
  The per-batch body runs under a single hardware loop (tc.For_i) so the
  static instruction count stays ~100 (walrus/NEFF compile time per call
  scales with module size, which dominates the measured dispatch delta).

Layout per batch element:
  nodes padded 150 -> 168 = 4 groups x 42; partition dim carries
  4 x (30|15|6)-feature groups; free dim carries (q, s) edge positions in
  14 chunks of 450 = 3 q-blocks x 150 senders.
"""

import numpy as np

import concourse.bacc as bacc
import concourse.tile as tile
from concourse import mybir
from concourse.bass import ds, ts

f32 = mybir.dt.float32
bf16 = mybir.dt.bfloat16
Alu = mybir.AluOpType
Act = mybir.ActivationFunctionType
AxX = mybir.AxisListType.X

# ---- problem dims (hardcoded per contract) ----
B, N, F = 128, 150, 16
NCORES = 8
BL = B // NCORES          # 16 batch elements per core
EH, EH2, NEFF = 30, 15, 6
DH, DH2, NDYN = 45, 22, 6
ABS, NCLS = 48, 5
BN_EPS = 1e-3
NP = 168                  # padded nodes = 4 * 42
QG = NP // 4              # 42 q positions per partition group
NCH = 14                  # chunks of 450 = 3 q-blocks
CHW = 450

# packed f32 const columns
CF_W = 136
# packed bf16 const columns
CB_W = 288


def _build_module():
    nc = bacc.Bacc("TRN2", target_bir_lowering=False)

    xt_d = nc.dram_tensor("xt", [BL * F, NP], bf16, kind="ExternalInput")
    cb_d = nc.dram_tensor("cb", [128, CB_W], bf16, kind="ExternalInput")
    cf_d = nc.dram_tensor("cf", [128, CF_W], f32, kind="ExternalInput")
    y_d = nc.dram_tensor("y", [BL, NCLS], f32, kind="ExternalOutput")

    from contextlib import ExitStack
    ctx = ExitStack()
    with tile.TileContext(nc) as tc, ctx:
        consts = ctx.enter_context(tc.tile_pool(name="consts", bufs=1))
        xp = ctx.enter_context(tc.tile_pool(name="xp", bufs=2))
        asp = ctx.enter_context(tc.tile_pool(name="asp", bufs=2))
        h1p = ctx.enter_context(tc.tile_pool(name="h1p", bufs=1))
        h2p = ctx.enter_context(tc.tile_pool(name="h2p", bufs=3))
        h3p = ctx.enter_context(tc.tile_pool(name="h3p", bufs=3))
        rp = ctx.enter_context(tc.tile_pool(name="rp", bufs=8))
        smp = ctx.enter_context(tc.tile_pool(name="smp", bufs=3))
        psAS = ctx.enter_context(tc.tile_pool(name="psAS", bufs=2, space="PSUM"))
        ps2 = ctx.enter_context(tc.tile_pool(name="ps2", bufs=2, space="PSUM"))
        ps3 = ctx.enter_context(tc.tile_pool(name="ps3", bufs=2, space="PSUM"))
        psD = ctx.enter_context(tc.tile_pool(name="psD", bufs=2, space="PSUM"))

        # ---- load packed constants ----
        cb = consts.tile([128, CB_W], bf16)
        cf = consts.tile([128, CF_W], f32)
        nc.sync.dma_start(out=cb, in_=cb_d.ap())
        nc.sync.dma_start(out=cf, in_=cf_d.ap())

        wr = cb[0:F, 0:32]
        ws = cb[0:F, 32:160]
        w2 = cb[:, 160:224]
        w3 = cb[:, 224:288]
        bA = cf[:, 0:1]
        bS = cf[:, 1:2]
        eb2r = cf[:, 2:3]
        eb3r = cf[:, 3:4]
        db1 = cf[0:DH, 4:5]
        db2 = cf[0:DH2, 5:6]
        db3 = cf[0:NDYN, 6:7]
        ab1 = cf[0:ABS, 7:8]
        sc_pp = cf[0:F, 8:9]
        sh_pp = cf[0:F, 9:10]
        w1d = cf[0:F + NEFF, 10:55]
        wd2 = cf[0:DH, 55:77]
        wd3 = cf[0:DH2, 77:83]
        wa1 = cf[0:NDYN, 83:131]
        wa2 = cf[0:ABS + 1, 131:136]

        pooled = consts.tile([NDYN, BL], f32)

        with tc.For_i(0, BL, 1) as b:
            x_tb = xp.tile([F, NP], bf16, tag="x_tb")
            nc.sync.dma_start(out=x_tb, in_=xt_d[ts(b, F)])

            # A[32j+f, q] = xn[42j+q] @ W1r'  (4 matmuls, M=32 col-tiles)
            a_ps_full = psAS.tile([128, 512], f32, tag="as")
            a_ps = a_ps_full[:, 0:QG]
            for j in range(4):
                nc.tensor.matmul(
                    out=a_ps[32 * j:32 * j + 32, :], lhsT=wr,
                    rhs=x_tb[:, QG * j:QG * (j + 1)],
                    start=True, stop=True, tile_position=(0, 32 * j))
            a_s = asp.tile([128, QG], bf16, tag="a_s")
            nc.scalar.activation(out=a_s, in_=a_ps, func=Act.Identity, bias=bA)

            # S_rep[32j+f, s] = xn[s] @ W1s' (one matmul, 4x replicated lhsT)
            s_ps_full = psAS.tile([128, 512], f32, tag="as")
            s_ps = s_ps_full[:, 0:N]
            nc.tensor.matmul(out=s_ps, lhsT=ws, rhs=x_tb[:, 0:N],
                             start=True, stop=True)
            s_rep = asp.tile([128, N], bf16, tag="s_rep")
            nc.scalar.activation(out=s_rep, in_=s_ps, func=Act.Identity, bias=bS)

            # h1[p, q*150+s] = relu(S_rep[p, s] + A[p, q]); zero the s==r diag
            h1 = h1p.tile([128, QG * N], bf16, tag="h1")
            nc.vector.tensor_tensor(
                out=h1[:, :].rearrange("p (q s) -> p q s", s=N),
                in0=s_rep[:, 0:N].unsqueeze(1).broadcast_to([128, QG, N]),
                in1=a_s[:, 0:QG].unsqueeze(2).broadcast_to([128, QG, N]),
                op=Alu.add)
            nc.vector.tensor_scalar_max(out=h1[:, :], in0=h1[:, :], scalar1=0.0)
            for j in range(4):
                nq = QG if j < 3 else N - 3 * QG
                nc.vector.memset(
                    h1[32 * j:32 * j + 32,
                       QG * j:QG * j + (N + 1) * (nq - 1) + 1:N + 1], 0.0)

            # stage 2 + 3; segmented sums via tensor_reduce
            effr = smp.tile([24, QG], f32, tag="effr")
            p3_cur = None
            h2_cur = None
            p2_cur = None
            for c in range(NCH):
                half = c % 2
                if half == 0:
                    p2_full = ps2.tile([128, 512], f32, tag="p2")
                    p2_cur = p2_full[:, 0:CHW]
                nc.tensor.matmul(
                    out=p2_cur[64 * half:64 * half + 64, :], lhsT=w2,
                    rhs=h1[:, c * CHW:(c + 1) * CHW],
                    start=True, stop=True, tile_position=(0, 64 * half))
                if half == 1:
                    p = c // 2
                    h2_cur = h2p.tile([128, CHW], bf16, tag="h2")
                    nc.vector.tensor_scalar(
                        out=h2_cur, in0=p2_cur, scalar1=eb2r,
                        scalar2=0.0, op0=Alu.add, op1=Alu.max)
                    g = p % 2
                    if g == 0:
                        p3_full = ps3.tile([128, 512], f32, tag="p3")
                        p3_cur = p3_full[:, 0:CHW]
                    nc.tensor.matmul(
                        out=p3_cur[64 * g:64 * g + 64, 0:CHW], lhsT=w3,
                        rhs=h2_cur, start=True, stop=True,
                        tile_position=(0, 64 * g))
                    if g == 1 or p == 6:
                        rows3 = 128 if g == 1 else 64
                        h3 = h3p.tile([128, CHW], f32, tag="h3")
                        nc.vector.tensor_scalar(
                            out=h3[0:rows3, :], in0=p3_cur[0:rows3, :],
                            scalar1=eb3r[0:rows3], scalar2=0.0,
                            op0=Alu.add, op1=Alu.max)
                        r_t = rp.tile([128, 3], f32, tag="r")
                        nc.vector.tensor_reduce(
                            out=r_t[0:rows3, :],
                            in_=h3[0:rows3, :].rearrange(
                                "p (i s) -> p i s", s=N),
                            axis=AxX, op=Alu.add)
                        # scatter rows (64g'+32u+6j+c) -> effr[6j+c, 3*ch+i]
                        p0 = p - 1 if g == 1 else p
                        for gg in range(rows3 // 64):
                            for u in range(2):
                                ch = 2 * (p0 + gg) + u
                                nc.sync.dma_start(
                                    out=effr[:, 3 * ch:3 * ch + 3],
                                    in_=r_t[64 * gg + 32 * u:
                                            64 * gg + 32 * u + 24, :])

            # dynamics MLP: rhsD = [xn ; EffR^T-layout], one K=22 matmul
            rhsD = smp.tile([F + NEFF, NP], f32, tag="rhsD")
            nc.vector.tensor_scalar(out=rhsD[0:F, :], in0=x_tb, scalar1=sc_pp,
                                    scalar2=sh_pp, op0=Alu.mult, op1=Alu.add)
            for j in range(4):
                nc.sync.dma_start(
                    out=rhsD[F:F + NEFF, QG * j:QG * (j + 1)],
                    in_=effr[NEFF * j:NEFF * (j + 1), :])
            d1_full = psD.tile([DH, 512], f32, tag="dps")
            d1 = d1_full[:, 0:N]
            nc.tensor.matmul(out=d1, lhsT=w1d, rhs=rhsD[:, 0:N],
                             start=True, stop=True)
            d1s = smp.tile([DH, N], f32, tag="d1s")
            nc.scalar.activation(out=d1s, in_=d1, func=Act.Relu, bias=db1)
            d2_full = psD.tile([DH2, 512], f32, tag="dps")
            d2 = d2_full[:, 0:N]
            nc.tensor.matmul(out=d2, lhsT=wd2, rhs=d1s, start=True, stop=True)
            d2s = smp.tile([DH2, N], f32, tag="d2s")
            nc.scalar.activation(out=d2s, in_=d2, func=Act.Relu, bias=db2)
            d3_full = psD.tile([NDYN, 512], f32, tag="dps")
            d3 = d3_full[:, 0:N]
            nc.tensor.matmul(out=d3, lhsT=wd3, rhs=d2s, start=True, stop=True)
            dyn_s = smp.tile([NDYN, N], f32, tag="dyn_s")
            nc.scalar.activation(out=dyn_s, in_=d3, func=Act.Relu, bias=db3,
                                 accum_out=pooled[:, ds(b, 1)])

        # abstract MLP + softmax (once per core)
        pa_full = psD.tile([ABS, 512], f32, tag="dps")
        pa = pa_full[:, 0:BL]
        nc.tensor.matmul(out=pa, lhsT=wa1, rhs=pooled, start=True, stop=True)
        ha = consts.tile([ABS + 1, BL], f32)
        nc.vector.memset(ha[:, :], 1.0)
        nc.scalar.activation(out=ha[0:ABS, :], in_=pa, func=Act.Relu, bias=ab1)
        zl_full = psD.tile([BL, 512], f32, tag="dps")
        zl = zl_full[:, 0:NCLS]
        nc.tensor.matmul(out=zl, lhsT=ha, rhs=wa2, start=True, stop=True)
        ex = consts.tile([BL, NCLS], f32)
        nc.scalar.activation(out=ex, in_=zl, func=Act.Exp)
        ssum = consts.tile([BL, 1], f32)
        nc.vector.tensor_reduce(out=ssum, in_=ex, axis=AxX, op=Alu.add)
        rcp = consts.tile([BL, 1], f32)
        nc.vector.reciprocal(out=rcp, in_=ssum)
        outt = consts.tile([BL, NCLS], f32)
        nc.vector.tensor_scalar_mul(out=outt, in0=ex, scalar1=rcp)
        nc.sync.dma_start(out=y_d.ap(), in_=outt)

    nc.compile()
    return nc


def _prep_consts(inp):
    """Host-side weight preprocessing (tiny, O(KB))."""
    g = lambda k: np.asarray(inp[k], np.float32)
    sc = g("bn_gamma") / np.sqrt(g("bn_var") + BN_EPS)
    sh = g("bn_beta") - g("bn_mean") * sc
    W1 = g("eW1")
    W1r = sc[:, None] * W1[:F]
    W1s = sc[:, None] * W1[F:]
    bA = sh @ W1[:F] + g("eb1")
    bS = sh @ W1[F:]

    cb = np.zeros((128, CB_W), np.float32)
    cb[:F, 0:EH] = W1r
    for j in range(4):
        cb[:F, 32 + 32 * j:32 + 32 * j + EH] = W1s
    eW2 = g("eW2")
    for j in range(4):
        cb[32 * j:32 * j + EH, 160 + 15 * j:160 + 15 * j + EH2] = eW2
    eW3 = g("eW3")
    for u in range(2):
        for j in range(4):
            cb[64 * u + 15 * j:64 * u + 15 * j + EH2,
               224 + 32 * u + 6 * j:224 + 32 * u + 6 * j + NEFF] = eW3

    # edge-MLP(0): constant self-edge effect, folded into dynamics b1
    E0 = np.maximum(np.maximum(g("eb2"), 0.0) @ eW3 + g("eb3"), 0.0)
    db1p = g("db1") - E0 @ g("dW1")[F:F + NEFF]

    cf = np.zeros((128, CF_W), np.float32)
    for j in range(4):
        cf[32 * j:32 * j + EH, 0] = bA
        cf[32 * j:32 * j + EH, 1] = bS
        cf[15 * j:15 * j + EH2, 2] = g("eb2")
        cf[64 + 15 * j:64 + 15 * j + EH2, 2] = g("eb2")
        for gg in range(2):
            for u in range(2):
                cf[64 * gg + 32 * u + 6 * j:64 * gg + 32 * u + 6 * j + NEFF,
                   3] = g("eb3")
    cf[0:DH, 4] = db1p
    cf[0:DH2, 5] = g("db2")
    cf[0:NDYN, 6] = g("db3")
    cf[0:ABS, 7] = g("ab1")
    cf[0:F, 8] = sc
    cf[0:F, 9] = sh
    cf[0:F + NEFF, 10:55] = g("dW1")
    cf[0:DH, 55:77] = g("dW2")
    cf[0:DH2, 77:83] = g("dW3")
    cf[0:NDYN, 83:131] = g("aW1")
    cf[0:ABS, 131:136] = g("aW2")
    cf[ABS, 131:136] = g("ab2")

    import ml_dtypes
    return {"cb": cb.astype(ml_dtypes.bfloat16), "cf": cf}


def _prep_xt(x):
    """x (B, N, F) -> per-core transposed/padded (NCORES, BL*F, NP) bf16."""
    import ml_dtypes
    x = np.asarray(x, np.float32)
    xt = np.zeros((B, F, NP), np.float32)
    xt[:, :, :N] = np.transpose(x, (0, 2, 1))
    return xt.reshape(NCORES, BL * F, NP).astype(ml_dtypes.bfloat16)


_NC_CACHE = {}


def _get_module():
    if "nc" not in _NC_CACHE:
        _NC_CACHE["nc"] = _build_module()
    return _NC_CACHE["nc"]


def make_in_maps(inputs):
    consts = _prep_consts(inputs)
    xt = _prep_xt(inputs["x"])
    return [dict(consts, xt=np.ascontiguousarray(xt[c])) for c in range(NCORES)]


def kernel(**inputs) -> np.ndarray:
    from concourse.bass_utils import run_bass_kernel_spmd
    nc = _get_module()
    in_maps = make_in_maps(inputs)
    res = run_bass_kernel_spmd(nc, in_maps, core_ids=list(range(NCORES)))
    return np.concatenate([r["y"] for r in res.results], axis=0)


# revision 17
# speedup vs baseline: 276.8638x; 8.5911x over previous
"""ConvIntNet (interaction-network) Trainium2 kernel.

Strategy (pure data parallelism over batch, 8 cores x 16 batch elements):
  The dense one-hot relation einsums are algebraically removed. With edges
  ordered receiver-major, edge (r, s) has
      h1 = relu(A[r] + S[s] + eb1),  A = xn @ W1_rec, S = xn @ W1_snd
  computed as ONE broadcast tensor_tensor op over a [128, 42*150] tile
  (4 node groups packed in the partition dim). Self-edges are removed by
  zeroing the h1 diagonal; the resulting constant edge-MLP(0) bias is
  folded into the dynamics b1 on the host. Stages 2/3 are block-diagonal
  packed matmuls; the receiver scatter-add is a segmented tensor_reduce.
  The per-batch body runs under a single hardware loop (tc.For_i) so the
  static instruction count stays ~100 (walrus/NEFF compile time per call
  scales with module size, which dominates the measured dispatch delta).

Layout per batch element:
  nodes padded 150 -> 168 = 4 groups x 42; partition dim carries
  4 x (30|15|6)-feature groups; free dim carries (q, s) edge positions in
  14 chunks of 450 = 3 q-blocks x 150 senders.
"""

import numpy as np

import concourse.bacc as bacc
import concourse.tile as tile
from concourse import mybir
from concourse.bass import ds, ts

f32 = mybir.dt.float32
bf16 = mybir.dt.bfloat16
Alu = mybir.AluOpType
Act = mybir.ActivationFunctionType
AxX = mybir.AxisListType.X

# ---- problem dims (hardcoded per contract) ----
B, N, F = 128, 150, 16
NCORES = 8
BL = B // NCORES          # 16 batch elements per core
EH, EH2, NEFF = 30, 15, 6
DH, DH2, NDYN = 45, 22, 6
ABS, NCLS = 48, 5
BN_EPS = 1e-3
NP = 168                  # padded nodes = 4 * 42
QG = NP // 4              # 42 q positions per partition group
NCH = 14                  # chunks of 450 = 3 q-blocks
CHW = 450

# packed f32 const columns
CF_W = 136
# packed bf16 const columns
CB_W = 288


def _build_module(rep=1):
    """Build the module. rep>1 wraps the batch loop in an outer repeat loop
    (idempotent re-execution) — used by test.py's slope-based HW timing."""
    nc = bacc.Bacc("TRN2", target_bir_lowering=False)

    xt_d = nc.dram_tensor("xt", [BL * F, NP], bf16, kind="ExternalInput")
    cb_d = nc.dram_tensor("cb", [128, CB_W], bf16, kind="ExternalInput")
    cf_d = nc.dram_tensor("cf", [128, CF_W], f32, kind="ExternalInput")
    y_d = nc.dram_tensor("y", [BL, NCLS], f32, kind="ExternalOutput")

    from contextlib import ExitStack
    ctx = ExitStack()
    with tile.TileContext(nc) as tc, ctx:
        consts = ctx.enter_context(tc.tile_pool(name="consts", bufs=1))
        xp = ctx.enter_context(tc.tile_pool(name="xp", bufs=2))
        asp = ctx.enter_context(tc.tile_pool(name="asp", bufs=2))
        h1p = ctx.enter_context(tc.tile_pool(name="h1p", bufs=1))
        h2p = ctx.enter_context(tc.tile_pool(name="h2p", bufs=3))
        h3p = ctx.enter_context(tc.tile_pool(name="h3p", bufs=3))
        rp = ctx.enter_context(tc.tile_pool(name="rp", bufs=8))
        smp = ctx.enter_context(tc.tile_pool(name="smp", bufs=3))
        psAS = ctx.enter_context(tc.tile_pool(name="psAS", bufs=2, space="PSUM"))
        ps2 = ctx.enter_context(tc.tile_pool(name="ps2", bufs=2, space="PSUM"))
        ps3 = ctx.enter_context(tc.tile_pool(name="ps3", bufs=2, space="PSUM"))
        psD = ctx.enter_context(tc.tile_pool(name="psD", bufs=2, space="PSUM"))

        # ---- load packed constants ----
        cb = consts.tile([128, CB_W], bf16)
        cf = consts.tile([128, CF_W], f32)
        nc.sync.dma_start(out=cb, in_=cb_d.ap())
        nc.sync.dma_start(out=cf, in_=cf_d.ap())

        wr = cb[0:F, 0:32]
        ws = cb[0:F, 32:160]
        w2 = cb[:, 160:224]
        w3 = cb[:, 224:288]
        bA = cf[:, 0:1]
        bS = cf[:, 1:2]
        eb2r = cf[:, 2:3]
        eb3r = cf[:, 3:4]
        db1 = cf[0:DH, 4:5]
        db2 = cf[0:DH2, 5:6]
        db3 = cf[0:NDYN, 6:7]
        ab1 = cf[0:ABS, 7:8]
        sc_pp = cf[0:F, 8:9]
        sh_pp = cf[0:F, 9:10]
        w1d = cf[0:F + NEFF, 10:55]
        wd2 = cf[0:DH, 55:77]
        wd3 = cf[0:DH2, 77:83]
        wa1 = cf[0:NDYN, 83:131]
        wa2 = cf[0:ABS + 1, 131:136]

        pooled = consts.tile([NDYN, BL], f32)

        from contextlib import nullcontext
        with (tc.For_i(0, rep, 1) if rep > 1 else nullcontext()), \
                tc.For_i(0, BL, 1) as b:
            x_tb = xp.tile([F, NP], bf16, tag="x_tb")
            nc.sync.dma_start(out=x_tb, in_=xt_d[ts(b, F)])

            # A[32j+f, q] = xn[42j+q] @ W1r'  (4 matmuls, M=32 col-tiles)
            a_ps_full = psAS.tile([128, 512], f32, tag="as")
            a_ps = a_ps_full[:, 0:QG]
            for j in range(4):
                nc.tensor.matmul(
                    out=a_ps[32 * j:32 * j + 32, :], lhsT=wr,
                    rhs=x_tb[:, QG * j:QG * (j + 1)],
                    start=True, stop=True, tile_position=(0, 32 * j))
            a_s = asp.tile([128, QG], bf16, tag="a_s")
            nc.vector.tensor_scalar(out=a_s, in0=a_ps, scalar1=bA,
                                    scalar2=None, op0=Alu.add)

            # S_rep[32j+f, s] = xn[s] @ W1s' (one matmul, 4x replicated lhsT)
            s_ps_full = psAS.tile([128, 512], f32, tag="as")
            s_ps = s_ps_full[:, 0:N]
            nc.tensor.matmul(out=s_ps, lhsT=ws, rhs=x_tb[:, 0:N],
                             start=True, stop=True)
            s_rep = asp.tile([128, N], bf16, tag="s_rep")
            nc.vector.tensor_scalar(out=s_rep, in0=s_ps, scalar1=bS,
                                    scalar2=None, op0=Alu.add)

            # h1[p, q*150+s] = relu(S_rep[p, s] + A[p, q]); zero the s==r diag
            h1 = h1p.tile([128, QG * N], bf16, tag="h1")
            nc.vector.tensor_tensor(
                out=h1[:, :].rearrange("p (q s) -> p q s", s=N),
                in0=s_rep[:, 0:N].unsqueeze(1).broadcast_to([128, QG, N]),
                in1=a_s[:, 0:QG].unsqueeze(2).broadcast_to([128, QG, N]),
                op=Alu.add)
            nc.vector.tensor_scalar_max(out=h1[:, :], in0=h1[:, :], scalar1=0.0)
            for j in range(4):
                nq = QG if j < 3 else N - 3 * QG
                nc.vector.memset(
                    h1[32 * j:32 * j + 32,
                       QG * j:QG * j + (N + 1) * (nq - 1) + 1:N + 1], 0.0)

            # stage 2 + 3; segmented sums via tensor_reduce
            effr = smp.tile([24, QG], f32, tag="effr")
            p3_cur = None
            h2_cur = None
            p2_cur = None
            for c in range(NCH):
                half = c % 2
                if half == 0:
                    p2_full = ps2.tile([128, 512], f32, tag="p2")
                    p2_cur = p2_full[:, 0:CHW]
                nc.tensor.matmul(
                    out=p2_cur[64 * half:64 * half + 64, :], lhsT=w2,
                    rhs=h1[:, c * CHW:(c + 1) * CHW],
                    start=True, stop=True, tile_position=(0, 64 * half))
                if half == 1:
                    p = c // 2
                    h2_cur = h2p.tile([128, CHW], bf16, tag="h2")
                    nc.vector.tensor_scalar(
                        out=h2_cur, in0=p2_cur, scalar1=eb2r,
                        scalar2=0.0, op0=Alu.add, op1=Alu.max)
                    g = p % 2
                    if g == 0:
                        p3_full = ps3.tile([128, 512], f32, tag="p3")
                        p3_cur = p3_full[:, 0:CHW]
                    nc.tensor.matmul(
                        out=p3_cur[64 * g:64 * g + 64, 0:CHW], lhsT=w3,
                        rhs=h2_cur, start=True, stop=True,
                        tile_position=(0, 64 * g))
                    if g == 1 or p == 6:
                        rows3 = 128 if g == 1 else 64
                        h3 = h3p.tile([128, CHW], f32, tag="h3")
                        nc.vector.tensor_scalar(
                            out=h3[0:rows3, :], in0=p3_cur[0:rows3, :],
                            scalar1=eb3r[0:rows3], scalar2=0.0,
                            op0=Alu.add, op1=Alu.max)
                        r_t = rp.tile([128, 3], f32, tag="r")
                        nc.vector.tensor_reduce(
                            out=r_t[0:rows3, :],
                            in_=h3[0:rows3, :].rearrange(
                                "p (i s) -> p i s", s=N),
                            axis=AxX, op=Alu.add)
                        # scatter rows (64g'+32u+6j+c) -> effr[6j+c, 3*ch+i]
                        p0 = p - 1 if g == 1 else p
                        for gg in range(rows3 // 64):
                            for u in range(2):
                                ch = 2 * (p0 + gg) + u
                                nc.sync.dma_start(
                                    out=effr[:, 3 * ch:3 * ch + 3],
                                    in_=r_t[64 * gg + 32 * u:
                                            64 * gg + 32 * u + 24, :])

            # dynamics MLP: rhsD = [xn ; EffR^T-layout], one K=22 matmul
            rhsD = smp.tile([F + NEFF, NP], f32, tag="rhsD")
            nc.vector.tensor_scalar(out=rhsD[0:F, :], in0=x_tb, scalar1=sc_pp,
                                    scalar2=sh_pp, op0=Alu.mult, op1=Alu.add)
            for j in range(4):
                nc.sync.dma_start(
                    out=rhsD[F:F + NEFF, QG * j:QG * (j + 1)],
                    in_=effr[NEFF * j:NEFF * (j + 1), :])
            d1_full = psD.tile([DH, 512], f32, tag="dps")
            d1 = d1_full[:, 0:N]
            nc.tensor.matmul(out=d1, lhsT=w1d, rhs=rhsD[:, 0:N],
                             start=True, stop=True)
            d1s = smp.tile([DH, N], f32, tag="d1s")
            nc.vector.tensor_scalar(out=d1s, in0=d1, scalar1=db1,
                                    scalar2=0.0, op0=Alu.add, op1=Alu.max)
            d2_full = psD.tile([DH2, 512], f32, tag="dps")
            d2 = d2_full[:, 0:N]
            nc.tensor.matmul(out=d2, lhsT=wd2, rhs=d1s, start=True, stop=True)
            d2s = smp.tile([DH2, N], f32, tag="d2s")
            nc.vector.tensor_scalar(out=d2s, in0=d2, scalar1=db2,
                                    scalar2=0.0, op0=Alu.add, op1=Alu.max)
            d3_full = psD.tile([NDYN, 512], f32, tag="dps")
            d3 = d3_full[:, 0:N]
            nc.tensor.matmul(out=d3, lhsT=wd3, rhs=d2s, start=True, stop=True)
            dyn_s = smp.tile([NDYN, N], f32, tag="dyn_s")
            nc.scalar.activation(out=dyn_s, in_=d3, func=Act.Relu, bias=db3,
                                 accum_out=pooled[:, ds(b, 1)])

        # abstract MLP + softmax (once per core)
        pa_full = psD.tile([ABS, 512], f32, tag="dps")
        pa = pa_full[:, 0:BL]
        nc.tensor.matmul(out=pa, lhsT=wa1, rhs=pooled, start=True, stop=True)
        ha = consts.tile([ABS + 1, BL], f32)
        nc.vector.memset(ha[:, :], 1.0)
        nc.scalar.activation(out=ha[0:ABS, :], in_=pa, func=Act.Relu, bias=ab1)
        zl_full = psD.tile([BL, 512], f32, tag="dps")
        zl = zl_full[:, 0:NCLS]
        nc.tensor.matmul(out=zl, lhsT=ha, rhs=wa2, start=True, stop=True)
        ex = consts.tile([BL, NCLS], f32)
        nc.scalar.activation(out=ex, in_=zl, func=Act.Exp)
        ssum = consts.tile([BL, 1], f32)
        nc.vector.tensor_reduce(out=ssum, in_=ex, axis=AxX, op=Alu.add)
        rcp = consts.tile([BL, 1], f32)
        nc.vector.reciprocal(out=rcp, in_=ssum)
        outt = consts.tile([BL, NCLS], f32)
        nc.vector.tensor_scalar_mul(out=outt, in0=ex, scalar1=rcp)
        nc.sync.dma_start(out=y_d.ap(), in_=outt)

    nc.compile()
    return nc


def _prep_consts(inp):
    """Host-side weight preprocessing (tiny, O(KB))."""
    g = lambda k: np.asarray(inp[k], np.float32)
    sc = g("bn_gamma") / np.sqrt(g("bn_var") + BN_EPS)
    sh = g("bn_beta") - g("bn_mean") * sc
    W1 = g("eW1")
    W1r = sc[:, None] * W1[:F]
    W1s = sc[:, None] * W1[F:]
    bA = sh @ W1[:F] + g("eb1")
    bS = sh @ W1[F:]

    cb = np.zeros((128, CB_W), np.float32)
    cb[:F, 0:EH] = W1r
    for j in range(4):
        cb[:F, 32 + 32 * j:32 + 32 * j + EH] = W1s
    eW2 = g("eW2")
    for j in range(4):
        cb[32 * j:32 * j + EH, 160 + 15 * j:160 + 15 * j + EH2] = eW2
    eW3 = g("eW3")
    for u in range(2):
        for j in range(4):
            cb[64 * u + 15 * j:64 * u + 15 * j + EH2,
               224 + 32 * u + 6 * j:224 + 32 * u + 6 * j + NEFF] = eW3

    # edge-MLP(0): constant self-edge effect, folded into dynamics b1
    E0 = np.maximum(np.maximum(g("eb2"), 0.0) @ eW3 + g("eb3"), 0.0)
    db1p = g("db1") - E0 @ g("dW1")[F:F + NEFF]

    cf = np.zeros((128, CF_W), np.float32)
    for j in range(4):
        cf[32 * j:32 * j + EH, 0] = bA
        cf[32 * j:32 * j + EH, 1] = bS
        cf[15 * j:15 * j + EH2, 2] = g("eb2")
        cf[64 + 15 * j:64 + 15 * j + EH2, 2] = g("eb2")
        for gg in range(2):
            for u in range(2):
                cf[64 * gg + 32 * u + 6 * j:64 * gg + 32 * u + 6 * j + NEFF,
                   3] = g("eb3")
    cf[0:DH, 4] = db1p
    cf[0:DH2, 5] = g("db2")
    cf[0:NDYN, 6] = g("db3")
    cf[0:ABS, 7] = g("ab1")
    cf[0:F, 8] = sc
    cf[0:F, 9] = sh
    cf[0:F + NEFF, 10:55] = g("dW1")
    cf[0:DH, 55:77] = g("dW2")
    cf[0:DH2, 77:83] = g("dW3")
    cf[0:NDYN, 83:131] = g("aW1")
    cf[0:ABS, 131:136] = g("aW2")
    cf[ABS, 131:136] = g("ab2")

    import ml_dtypes
    return {"cb": cb.astype(ml_dtypes.bfloat16), "cf": cf}


def _prep_xt(x):
    """x (B, N, F) -> per-core transposed/padded (NCORES, BL*F, NP) bf16."""
    import ml_dtypes
    x = np.asarray(x, np.float32)
    xt = np.zeros((B, F, NP), np.float32)
    xt[:, :, :N] = np.transpose(x, (0, 2, 1))
    return xt.reshape(NCORES, BL * F, NP).astype(ml_dtypes.bfloat16)


_NC_CACHE = {}


def _get_module(rep=1):
    key = ("nc", rep)
    if key not in _NC_CACHE:
        _NC_CACHE[key] = _build_module(rep)
    return _NC_CACHE[key]


def make_in_maps(inputs):
    consts = _prep_consts(inputs)
    xt = _prep_xt(inputs["x"])
    return [dict(consts, xt=np.ascontiguousarray(xt[c])) for c in range(NCORES)]


def _build_executor(rep=1):
    """Compile the module once into a reusable sharded PJRT executable.

    run_bass_kernel_spmd builds a fresh jit closure per call, which re-runs
    jaxpr tracing, BIR serialization and the walrus NEFF compile every time
    (~200ms/call).  Caching the jitted callable makes repeat kernel() calls
    pure dispatch+execute, exactly like any compile-once/run-many kernel.
    """
    import jax
    from jax.sharding import Mesh, PartitionSpec
    from jax.experimental.shard_map import shard_map
    import concourse.bass2jax as b2j

    nc = _get_module(rep)
    b2j.install_neuronx_cc_hook()
    partition_name = (
        nc.partition_id_tensor.name if nc.partition_id_tensor else None)
    in_names, out_names, out_avals = [], [], []
    for alloc in nc.m.functions[0].allocations:
        if not isinstance(alloc, mybir.MemoryLocationSet):
            continue
        name = alloc.memorylocations[0].name
        if alloc.kind == "ExternalInput":
            if name != partition_name:
                in_names.append(name)
        elif alloc.kind == "ExternalOutput":
            shape = tuple(alloc.tensor_shape)
            dtype = mybir.dt.np(alloc.dtype)
            out_names.append(name)
            out_avals.append(jax.core.ShapedArray(shape, dtype))
    n_params = len(in_names)
    n_outs = len(out_avals)
    in_names_full = list(in_names) + out_names
    if partition_name is not None:
        in_names_full.append(partition_name)

    def _body(*args):
        operands = list(args)
        if partition_name is not None:
            operands.append(b2j.partition_id_tensor())
        outs = b2j._bass_exec_p.bind(
            *operands, out_avals=tuple(out_avals),
            in_names=tuple(in_names_full), out_names=tuple(out_names),
            lowering_input_output_aliases=(), sim_require_finite=True,
            sim_require_nnan=True, nc=nc)
        return tuple(outs)

    devices = jax.devices()[:NCORES]
    assert len(devices) == NCORES
    mesh = Mesh(np.asarray(devices), ("core",))
    donate = tuple(range(n_params, n_params + n_outs))
    sharded = jax.jit(
        shard_map(_body, mesh=mesh,
                  in_specs=(PartitionSpec("core"),) * (n_params + n_outs),
                  out_specs=(PartitionSpec("core"),) * n_outs,
                  check_rep=False),
        donate_argnums=donate, keep_unused=True)
    zero_shapes = [((NCORES * a.shape[0],) + tuple(a.shape[1:]), a.dtype)
                   for a in out_avals]

    def run(in_maps):
        concat_in = [
            np.concatenate([np.asarray(m[nm]) for m in in_maps], axis=0)
            for nm in in_names]
        concat_zeros = [np.zeros(s, d) for s, d in zero_shapes]
        outs = sharded(*concat_in, *concat_zeros)
        return {nm: np.asarray(outs[i]) for i, nm in enumerate(out_names)}

    return run


def _get_executor(rep=1):
    key = ("run", rep)
    if key not in _NC_CACHE:
        _NC_CACHE[key] = _build_executor(rep)
    return _NC_CACHE[key]


def kernel(**inputs) -> np.ndarray:
    run = _get_executor()
    in_maps = make_in_maps(inputs)
    out = run(in_maps)
    return out["y"].reshape(B, NCLS)


# revision 45
# speedup vs baseline: 594.8310x; 2.1485x over previous
"""ConvIntNet (interaction-network) Trainium2 kernel.

Strategy (pure data parallelism over batch, 8 cores x 16 batch elements):
  The dense one-hot relation einsums are algebraically removed. With edges
  ordered receiver-major, edge (r, s) has
      h1 = relu(A[r] + S[s] + eb1),  A = xn @ W1_rec, S = xn @ W1_snd
  computed as ONE broadcast tensor_tensor op over a [128, 42*150] tile
  (4 node groups packed in the partition dim). Self-edges are removed by
  zeroing the h1 diagonal; the resulting constant edge-MLP(0) bias is
  folded into the dynamics b1 on the host. Stages 2/3 are block-diagonal
  packed matmuls; the receiver scatter-add is a segmented tensor_reduce.
  The per-batch body runs under a single hardware loop (tc.For_i) so the
  static instruction count stays ~100 (walrus/NEFF compile time per call
  scales with module size, which dominates the measured dispatch delta).

Layout per batch element:
  nodes padded 150 -> 168 = 4 groups x 42; partition dim carries
  4 x (30|15|6)-feature groups; free dim carries (q, s) edge positions in
  14 chunks of 450 = 3 q-blocks x 150 senders.
"""

import numpy as np

import concourse.bacc as bacc
import concourse.tile as tile
from concourse import mybir
from concourse.bass import ds, ts

f32 = mybir.dt.float32
bf16 = mybir.dt.bfloat16
Alu = mybir.AluOpType
Act = mybir.ActivationFunctionType
AxX = mybir.AxisListType.X

# ---- problem dims (hardcoded per contract) ----
B, N, F = 128, 150, 16
NCORES = 8
BL = B // NCORES          # 16 batch elements per core
EH, EH2, NEFF = 30, 15, 6
DH, DH2, NDYN = 45, 22, 6
ABS, NCLS = 48, 5
BN_EPS = 1e-3
NP = 168                  # padded nodes = 4 * 42
QG = NP // 4              # 42 q positions per partition group
NCH = 14                  # chunks of 450 = 3 q-blocks
CHW = 450

# packed f32 const columns
CF_W = 136
# packed bf16 const columns
CB_W = 288

# loop structure: "for_i" | "staggered" | "unroll"
STRUCT = "unroll"


def _build_module(rep=1):
    """Build the module. rep>1 wraps the batch loop in an outer repeat loop
    (idempotent re-execution) — used by test.py's slope-based HW timing."""
    nc = bacc.Bacc("TRN2", target_bir_lowering=False)

    xt_d = nc.dram_tensor("xt", [BL * F, NP], bf16, kind="ExternalInput")
    cb_d = nc.dram_tensor("cb", [128, CB_W], bf16, kind="ExternalInput")
    cf_d = nc.dram_tensor("cf", [128, CF_W], f32, kind="ExternalInput")
    y_d = nc.dram_tensor("y", [BL, NCLS], f32, kind="ExternalOutput")

    from contextlib import ExitStack
    ctx = ExitStack()
    with tile.TileContext(nc) as tc, ctx:
        consts = ctx.enter_context(tc.tile_pool(name="consts", bufs=1))
        xp = ctx.enter_context(tc.tile_pool(name="xp", bufs=3))
        asp = ctx.enter_context(tc.tile_pool(name="asp", bufs=2))
        h1p = ctx.enter_context(tc.tile_pool(name="h1p", bufs=3))
        h2p = ctx.enter_context(tc.tile_pool(name="h2p", bufs=3))
        h3p = ctx.enter_context(tc.tile_pool(name="h3p", bufs=4))
        smp = ctx.enter_context(tc.tile_pool(name="smp", bufs=3))
        psAS = ctx.enter_context(tc.tile_pool(name="psAS", bufs=2, space="PSUM"))
        ps2 = ctx.enter_context(tc.tile_pool(name="ps2", bufs=2, space="PSUM"))
        ps3 = ctx.enter_context(tc.tile_pool(name="ps3", bufs=2, space="PSUM"))
        psD = ctx.enter_context(tc.tile_pool(name="psD", bufs=2, space="PSUM"))

        # ---- load packed constants ----
        cb = consts.tile([128, CB_W], bf16)
        cf = consts.tile([128, CF_W], f32)
        nc.sync.dma_start(out=cb, in_=cb_d.ap())
        nc.sync.dma_start(out=cf, in_=cf_d.ap())

        wr = cb[0:F, 0:32]
        ws = cb[0:F, 32:160]
        w2 = cb[:, 160:224]
        w3 = cb[:, 224:288]
        bA = cf[:, 0:1]
        bS = cf[:, 1:2]
        eb2r = cf[:, 2:3]
        eb3r = cf[:, 3:4]
        db1 = cf[0:DH, 4:5]
        db2 = cf[0:DH2, 5:6]
        db3 = cf[0:NDYN, 6:7]
        ab1 = cf[0:ABS, 7:8]
        sc_pp = cf[0:F, 8:9]
        sh_pp = cf[0:F, 9:10]
        w1d = cf[0:F + NEFF, 10:55]
        wd2 = cf[0:DH, 55:77]
        wd3 = cf[0:DH2, 77:83]
        wa1 = cf[0:NDYN, 83:131]
        wa2 = cf[0:ABS + 1, 131:136]

        pooled = consts.tile([NDYN, BL], f32)

        def _batch_body(b):
            x_tb = xp.tile([F, NP], bf16, tag="x_tb")
            nc.sync.dma_start(out=x_tb, in_=xt_d[ts(b, F)])

            # A[32j+f, q] = xn[42j+q] @ W1r'  (4 matmuls, M=32 col-tiles)
            a_ps_full = psAS.tile([128, 512], f32, tag="as")
            a_ps = a_ps_full[:, 0:QG]
            for j in range(4):
                nc.tensor.matmul(
                    out=a_ps[32 * j:32 * j + 32, :], lhsT=wr,
                    rhs=x_tb[:, QG * j:QG * (j + 1)],
                    start=True, stop=True, tile_position=(0, 32 * j))
            a_s = asp.tile([128, QG], bf16, tag="a_s")
            nc.scalar.activation(out=a_s, in_=a_ps, func=Act.Identity, bias=bA)

            # S_rep[32j+f, s] = xn[s] @ W1s' (one matmul, 4x replicated lhsT)
            s_ps_full = psAS.tile([128, 512], f32, tag="as")
            s_ps = s_ps_full[:, 0:N]
            nc.tensor.matmul(out=s_ps, lhsT=ws, rhs=x_tb[:, 0:N],
                             start=True, stop=True)
            s_rep = asp.tile([128, N], bf16, tag="s_rep")
            nc.scalar.activation(out=s_rep, in_=s_ps, func=Act.Identity,
                                 bias=bS)

            # h1[p, q*150+s] = relu(S_rep[p, s] + A[p, q]); zero the s==r diag
            h1 = h1p.tile([128, QG * N], bf16, tag="h1")
            nc.vector.tensor_tensor(
                out=h1[:, :].rearrange("p (q s) -> p q s", s=N),
                in0=s_rep[:, 0:N].unsqueeze(1).broadcast_to([128, QG, N]),
                in1=a_s[:, 0:QG].unsqueeze(2).broadcast_to([128, QG, N]),
                op=Alu.add)
            nc.vector.tensor_scalar_max(out=h1[:, :], in0=h1[:, :],
                                        scalar1=0.0)
            for j in range(4):
                nq = QG if j < 3 else N - 3 * QG
                nc.vector.memset(
                    h1[32 * j:32 * j + 32,
                       QG * j:QG * j + (N + 1) * (nq - 1) + 1:N + 1], 0.0)

            # stage 2 + 3; segmented sums via tensor_reduce
            effr = smp.tile([64, 21], f32, tag="effr")
            p3_cur = None
            h2_cur = None
            p2_cur = None
            for c in range(NCH):
                half = c % 2
                if half == 0:
                    p2_full = ps2.tile([128, 512], f32, tag="p2")
                    p2_cur = p2_full[:, 0:CHW]
                nc.tensor.matmul(
                    out=p2_cur[64 * half:64 * half + 64, :], lhsT=w2,
                    rhs=h1[:, c * CHW:(c + 1) * CHW],
                    start=True, stop=True, tile_position=(0, 64 * half))
                if half == 1 or c == NCH - 1:
                    p = c // 2
                    rows2 = 128 if half == 1 else 64
                    rows3 = 56 if half == 1 else 24
                    h2_cur = h2p.tile([128, CHW], bf16, tag="h2")
                    nc.scalar.activation(out=h2_cur[0:rows2, :],
                                         in_=p2_cur[0:rows2, :],
                                         func=Act.Relu, bias=eb2r[0:rows2])
                    p3_full = ps3.tile([64, 512], f32, tag="p3")
                    p3_cur = p3_full[:, 0:CHW]
                    nc.tensor.matmul(out=p3_cur, lhsT=w3[0:rows2, :],
                                     rhs=h2_cur[0:rows2, :],
                                     start=True, stop=True)
                    h3 = h3p.tile([64, CHW], f32, tag="h3")
                    nc.scalar.activation(out=h3[0:rows3, :],
                                         in_=p3_cur[0:rows3, :],
                                         func=Act.Relu, bias=eb3r[0:rows3])
                    # segmented sums -> effr64[32u+6j+c, 3p+i] directly
                    # (rows 24:32 are zero-weight padding; effr rows 24:32
                    # are never read)
                    nc.vector.tensor_reduce(
                        out=effr[0:rows3, 3 * p:3 * p + 3],
                        in_=h3[0:rows3, :].rearrange("p (i s) -> p i s", s=N),
                        axis=AxX, op=Alu.add)

            # dynamics MLP: rhsD = [xn ; EffR^T-layout], one K=22 matmul
            rhsD = smp.tile([F + NEFF, NP], f32, tag="rhsD")
            nc.vector.tensor_scalar(out=rhsD[0:F, :], in0=x_tb, scalar1=sc_pp,
                                    scalar2=sh_pp, op0=Alu.mult, op1=Alu.add)
            for j in range(4):
                for u in range(2):
                    dst = rhsD[F:F + NEFF, QG * j:QG * (j + 1)].rearrange(
                        "p (pp ui) -> p pp ui", ui=6)[:, :, 3 * u:3 * u + 3]
                    nc.sync.dma_start(
                        out=dst,
                        in_=effr[32 * u + NEFF * j:32 * u + NEFF * (j + 1),
                                 :].rearrange("p (pp i) -> p pp i", i=3))
            d1_full = psD.tile([DH, 512], f32, tag="dps")
            d1 = d1_full[:, 0:N]
            nc.tensor.matmul(out=d1, lhsT=w1d, rhs=rhsD[:, 0:N],
                             start=True, stop=True)
            d1s = smp.tile([DH, N], f32, tag="d1s")
            nc.scalar.activation(out=d1s, in_=d1, func=Act.Relu, bias=db1)
            d2_full = psD.tile([DH2, 512], f32, tag="dps")
            d2 = d2_full[:, 0:N]
            nc.tensor.matmul(out=d2, lhsT=wd2, rhs=d1s, start=True, stop=True)
            d2s = smp.tile([DH2, N], f32, tag="d2s")
            nc.scalar.activation(out=d2s, in_=d2, func=Act.Relu, bias=db2)
            d3_full = psD.tile([NDYN, 512], f32, tag="dps")
            d3 = d3_full[:, 0:N]
            nc.tensor.matmul(out=d3, lhsT=wd3, rhs=d2s, start=True, stop=True)
            dyn_s = smp.tile([NDYN, N], f32, tag="dyn_s")
            nc.scalar.activation(out=dyn_s, in_=d3, func=Act.Relu, bias=db3,
                                 accum_out=pooled[:, ds(b, 1)])

        from contextlib import nullcontext
        outer = tc.For_i(0, rep, 1) if rep > 1 else nullcontext()
        with outer:
            if STRUCT == "unroll":
                for bi in range(BL):
                    _batch_body(bi)
            elif STRUCT == "staggered":
                with tc.For_i(0, BL, 1, staggered_reset=True) as b:
                    _batch_body(b)
            else:
                with tc.For_i(0, BL, 1) as b:
                    _batch_body(b)

        # abstract MLP + softmax (once per core)
        pa_full = psD.tile([ABS, 512], f32, tag="dps")
        pa = pa_full[:, 0:BL]
        nc.tensor.matmul(out=pa, lhsT=wa1, rhs=pooled, start=True, stop=True)
        ha = consts.tile([ABS + 1, BL], f32)
        nc.vector.memset(ha[:, :], 1.0)
        nc.scalar.activation(out=ha[0:ABS, :], in_=pa, func=Act.Relu, bias=ab1)
        zl_full = psD.tile([BL, 512], f32, tag="dps")
        zl = zl_full[:, 0:NCLS]
        nc.tensor.matmul(out=zl, lhsT=ha, rhs=wa2, start=True, stop=True)
        ex = consts.tile([BL, NCLS], f32)
        nc.scalar.activation(out=ex, in_=zl, func=Act.Exp)
        ssum = consts.tile([BL, 1], f32)
        nc.vector.tensor_reduce(out=ssum, in_=ex, axis=AxX, op=Alu.add)
        rcp = consts.tile([BL, 1], f32)
        nc.vector.reciprocal(out=rcp, in_=ssum)
        outt = consts.tile([BL, NCLS], f32)
        nc.vector.tensor_scalar_mul(out=outt, in0=ex, scalar1=rcp)
        nc.sync.dma_start(out=y_d.ap(), in_=outt)

    nc.compile()
    return nc


def _prep_consts(inp):
    """Host-side weight preprocessing (tiny, O(KB))."""
    g = lambda k: np.asarray(inp[k], np.float32)
    sc = g("bn_gamma") / np.sqrt(g("bn_var") + BN_EPS)
    sh = g("bn_beta") - g("bn_mean") * sc
    W1 = g("eW1")
    W1r = sc[:, None] * W1[:F]
    W1s = sc[:, None] * W1[F:]
    bA = sh @ W1[:F] + g("eb1")
    bS = sh @ W1[F:]

    cb = np.zeros((128, CB_W), np.float32)
    cb[:F, 0:EH] = W1r
    for j in range(4):
        cb[:F, 32 + 32 * j:32 + 32 * j + EH] = W1s
    eW2 = g("eW2")
    for j in range(4):
        cb[32 * j:32 * j + EH, 160 + 15 * j:160 + 15 * j + EH2] = eW2
    eW3 = g("eW3")
    for u in range(2):
        for j in range(4):
            cb[64 * u + 15 * j:64 * u + 15 * j + EH2,
               224 + 32 * u + 6 * j:224 + 32 * u + 6 * j + NEFF] = eW3

    # edge-MLP(0): constant self-edge effect, folded into dynamics b1
    E0 = np.maximum(np.maximum(g("eb2"), 0.0) @ eW3 + g("eb3"), 0.0)
    db1p = g("db1") - E0 @ g("dW1")[F:F + NEFF]

    cf = np.zeros((128, CF_W), np.float32)
    for j in range(4):
        cf[32 * j:32 * j + EH, 0] = bA
        cf[32 * j:32 * j + EH, 1] = bS
        cf[15 * j:15 * j + EH2, 2] = g("eb2")
        cf[64 + 15 * j:64 + 15 * j + EH2, 2] = g("eb2")
        for gg in range(2):
            for u in range(2):
                cf[64 * gg + 32 * u + 6 * j:64 * gg + 32 * u + 6 * j + NEFF,
                   3] = g("eb3")
    cf[0:DH, 4] = db1p
    cf[0:DH2, 5] = g("db2")
    cf[0:NDYN, 6] = g("db3")
    cf[0:ABS, 7] = g("ab1")
    cf[0:F, 8] = sc
    cf[0:F, 9] = sh
    cf[0:F + NEFF, 10:55] = g("dW1")
    cf[0:DH, 55:77] = g("dW2")
    cf[0:DH2, 77:83] = g("dW3")
    cf[0:NDYN, 83:131] = g("aW1")
    cf[0:ABS, 131:136] = g("aW2")
    cf[ABS, 131:136] = g("ab2")

    import ml_dtypes
    return {"cb": cb.astype(ml_dtypes.bfloat16), "cf": cf}


def _prep_xt(x):
    """x (B, N, F) -> per-core transposed/padded (NCORES, BL*F, NP) bf16."""
    import ml_dtypes
    x = np.asarray(x, np.float32)
    xt = np.zeros((B, F, NP), np.float32)
    xt[:, :, :N] = np.transpose(x, (0, 2, 1))
    return xt.reshape(NCORES, BL * F, NP).astype(ml_dtypes.bfloat16)


_NC_CACHE = {}


def _get_module(rep=1):
    key = ("nc", rep, STRUCT)
    if key not in _NC_CACHE:
        _NC_CACHE[key] = _build_module(rep)
    return _NC_CACHE[key]


def make_in_maps(inputs):
    consts = _prep_consts(inputs)
    xt = _prep_xt(inputs["x"])
    return [dict(consts, xt=np.ascontiguousarray(xt[c])) for c in range(NCORES)]


def _build_executor(rep=1):
    """Compile the module once into a reusable sharded PJRT executable.

    run_bass_kernel_spmd builds a fresh jit closure per call, which re-runs
    jaxpr tracing, BIR serialization and the walrus NEFF compile every time
    (~200ms/call).  Caching the jitted callable makes repeat kernel() calls
    pure dispatch+execute, exactly like any compile-once/run-many kernel.
    """
    import jax
    from jax.sharding import Mesh, PartitionSpec
    from jax.experimental.shard_map import shard_map
    import concourse.bass2jax as b2j

    nc = _get_module(rep)
    b2j.install_neuronx_cc_hook()
    partition_name = (
        nc.partition_id_tensor.name if nc.partition_id_tensor else None)
    in_names, out_names, out_avals = [], [], []
    for alloc in nc.m.functions[0].allocations:
        if not isinstance(alloc, mybir.MemoryLocationSet):
            continue
        name = alloc.memorylocations[0].name
        if alloc.kind == "ExternalInput":
            if name != partition_name:
                in_names.append(name)
        elif alloc.kind == "ExternalOutput":
            shape = tuple(alloc.tensor_shape)
            dtype = mybir.dt.np(alloc.dtype)
            out_names.append(name)
            out_avals.append(jax.core.ShapedArray(shape, dtype))
    n_params = len(in_names)
    n_outs = len(out_avals)
    in_names_full = list(in_names) + out_names
    if partition_name is not None:
        in_names_full.append(partition_name)

    def _body(*args):
        operands = list(args)
        if partition_name is not None:
            operands.append(b2j.partition_id_tensor())
        outs = b2j._bass_exec_p.bind(
            *operands, out_avals=tuple(out_avals),
            in_names=tuple(in_names_full), out_names=tuple(out_names),
            lowering_input_output_aliases=(), sim_require_finite=True,
            sim_require_nnan=True, nc=nc)
        return tuple(outs)

    devices = jax.devices()[:NCORES]
    assert len(devices) == NCORES
    mesh = Mesh(np.asarray(devices), ("core",))
    donate = tuple(range(n_params, n_params + n_outs))
    sharded = jax.jit(
        shard_map(_body, mesh=mesh,
                  in_specs=(PartitionSpec("core"),) * (n_params + n_outs),
                  out_specs=(PartitionSpec("core"),) * n_outs,
                  check_rep=False),
        donate_argnums=donate, keep_unused=True)
    zero_shapes = [((NCORES * a.shape[0],) + tuple(a.shape[1:]), a.dtype)
                   for a in out_avals]

    def run(in_maps):
        concat_in = [
            np.concatenate([np.asarray(m[nm]) for m in in_maps], axis=0)
            for nm in in_names]
        concat_zeros = [np.zeros(s, d) for s, d in zero_shapes]
        outs = sharded(*concat_in, *concat_zeros)
        return {nm: np.asarray(outs[i]) for i, nm in enumerate(out_names)}

    return run


def _get_executor(rep=1):
    key = ("run", rep, STRUCT)
    if key not in _NC_CACHE:
        _NC_CACHE[key] = _build_executor(rep)
    return _NC_CACHE[key]


def kernel(**inputs) -> np.ndarray:
    run = _get_executor()
    in_maps = make_in_maps(inputs)
    out = run(in_maps)
    return out["y"].reshape(B, NCLS)
